# revision 2
# baseline (speedup 1.0000x reference)
"""Trainium2 Bass kernel for a dense transformer attention block.

Reference (per batch b of 4, seq S=2048, embed E=1024, H=16 heads, D=64):
    q/k/v = x @ W{q,k,v}.T + b,  split heads
    attn  = softmax(q k^T / sqrt(D)),  ctx = attn @ v
    out   = LN(ctx @ Wo.T + bo + x) * ln_g + ln_b

Sharding (8 cores, no collectives): core c handles batch b=c//2 and query rows
[1024*(c%2), 1024*(c%2+1)).  Each core computes K/V projections for its full
batch (duplicated with its pair core, ~25% extra FLOPs, zero comms), attention
for all 16 heads over its 1024 query rows, out-projection + residual + LN for
its rows.  Host reassembles the 8 row-shards.

Core layout strategy:
  - scores computed TRANSPOSED: S^T[k, q] with K^T stationary / Q^T moving, so
    exp(S^T) feeds the ctx matmul directly as the moving operand - no PE
    transposes anywhere in the kernel.
  - QUAD-head q/k layout: 4 heads per 128 partitions, head-dim split as
    d = 32*plane + (p%32) with the plane in a free dim.  Scores then run as
    fp8 DoubleRow matmuls (Ki=32, Ko=2) at 0.5 cyc/row - 2x the throughput of
    the plain-fp8 two-head row-group scheme.
  - exp split between ACT (true exp) and DVE (Schraudolph: fp8 bit-trick
    u8 = rne(score*log2(e) + 56) bitcast as fp8e4m3 ~= exp(score/8)); both
    paths share scale so they mix freely within one softmax row.  The split
    ratio balances the two engines, which are otherwise the bottleneck.
  - softmax denominator from a ones-column appended to V (stationary [V_h|1],
    M=65): PSUM row 64 accumulates sum_k exp.  Normalization (x16 to keep fp8
    ctx out of the subnormal range; /16 folded into the out-proj epilogue) is
    fused into the PSUM->SBUF cast via partition_broadcast of the reciprocal.
  - ctx lands as ctx^T[e, q], exactly the stationary layout out-proj needs.
  - q/k/v/out projections and ctx run in fp8e4 with DoubleRow (0.5 cyc/row);
    PSUM accumulation is always fp32, residual + layernorm in fp32.  The
    attention branch contributes only ~0.8% of the output magnitude (the
    residual dominates), so fp8 path error dilutes ~128x in the final output.
  - scores+exp for unit i overlap ctx for unit i-1 (software pipeline), and
    the next quad's projection matmuls are interleaved into the attention
    stream so the in-order PE queue always has work.
"""

import sys

if "/opt/trn_rl_repo" not in sys.path:
    sys.path.insert(0, "/opt/trn_rl_repo")

import numpy as np
import ml_dtypes

B, S, E = 4, 2048, 1024
H, D = 16, 64
NQ = S // 2          # query rows per core
P = 128
ET = E // P          # 8 e-tiles
KT = S // P          # 16 k-tiles
W65 = D + 1          # V head width incl. ones column
VW = H * W65         # 1040
NCORES = 8
NQUAD = 4            # head quads (4 heads on 128 partitions)
CTX_SCALE = 16.0     # keep fp8 ctx in normal range

SCHRA_A = 1.4426950408889634   # log2(e): u8 = rne(A*s + B), bits = fp8e4m3
SCHRA_B = 56.0                 # exponent bias 7 * 8 mantissa steps

FP8 = ml_dtypes.float8_e4m3

_cache = {}

# k-tile grouping for exp batching: 3-bank PSUM groups amortize ACT/DVE
# per-instruction overhead over up to 1536 columns
GROUPS = [(0, 3), (3, 3), (6, 3), (9, 3), (12, 2), (14, 2)]


def _exp_engine(u, g):
    """Engine for the exp of unit u (0..15), group g (0..5): ACT or DVE.
    Balances ACT (true exp, 0.83 ns/el) against DVE (Schraudolph, 1.04
    ns/el); both are the runtime bottleneck so the split ratio matters."""
    if g >= 4:
        return "D"
    if g == 3 and u % 2 == 0:
        return "D"
    return "A"


def _build_nc(skip_affine=False, skip_bias=False):
    import concourse.bass as bass
    import concourse.tile as tile
    from concourse import bacc, mybir

    f8 = mybir.dt.float8e4
    f32 = mybir.dt.float32
    u8 = mybir.dt.uint8
    DR = mybir.MatmulPerfMode.DoubleRow

    nc = bacc.Bacc(None, target_bir_lowering=False, debug=False)

    d_xkT = nc.dram_tensor("xkT", [E, S], f8, kind="ExternalInput")
    d_xqT = nc.dram_tensor("xqT", [E, NQ], f8, kind="ExternalInput")
    d_xq = nc.dram_tensor("xq", [NQ, E], f32, kind="ExternalInput")
    # wq/wk pre-shuffled on host to [quad, p, plane, t, m] so each quad's
    # slice DMAs as contiguous 2KB runs; e_out(quad, plane, m) =
    # 64*(4*quad + m//32) + 32*plane + m%32 (the quad-DR scores layout)
    d_wqR = nc.dram_tensor("wqR", [NQUAD, P, 2, ET, P], f8, kind="ExternalInput")
    d_wkR = nc.dram_tensor("wkR", [NQUAD, P, 2, ET, P], f8, kind="ExternalInput")
    d_wvT = nc.dram_tensor("wvT", [E, E], f8, kind="ExternalInput")
    d_woT = nc.dram_tensor("woT", [E, E], f8, kind="ExternalInput")
    if not skip_bias:
        d_bq = nc.dram_tensor("bq", [E], f32, kind="ExternalInput")
        d_bk = nc.dram_tensor("bk", [E], f32, kind="ExternalInput")
        d_bv = nc.dram_tensor("bv", [E], f32, kind="ExternalInput")
    d_lng = nc.dram_tensor("lng", [E], f32, kind="ExternalInput")
    d_lnb = nc.dram_tensor("lnb", [E], f32, kind="ExternalInput")
    d_out = nc.dram_tensor("out", [NQ, E], f32, kind="ExternalOutput")

    def bcast_ap(d):
        ap = d[:]
        return bass.AP(tensor=ap.tensor, offset=ap.offset, ap=[[0, P], [1, E]])

    from contextlib import ExitStack

    with tile.TileContext(nc) as tc, ExitStack() as ctx:
        persist = ctx.enter_context(tc.tile_pool(name="persist", bufs=1))
        wslice = ctx.enter_context(tc.tile_pool(name="wslice", bufs=2))
        qkpool = ctx.enter_context(tc.tile_pool(name="qkpool", bufs=2))
        ppool = ctx.enter_context(tc.tile_pool(name="ppool", bufs=26))
        misc = ctx.enter_context(tc.tile_pool(name="misc", bufs=4))
        xqp_bufs = 6 if (skip_affine and skip_bias) else 4
        xqp = ctx.enter_context(tc.tile_pool(name="xqp", bufs=xqp_bufs))
        outp_bufs = 4 if (skip_affine and skip_bias) else 3
        outp = ctx.enter_context(tc.tile_pool(name="outp", bufs=outp_bufs))
        psum = ctx.enter_context(tc.tile_pool(name="psum", bufs=2, space="PSUM"))

        dma = nc.sync

        # ---- persistent tiles ----
        XK = persist.tile([P, ET, S], f8, tag="XK")       # x[b]^T, e-tiles on dim1
        XQ = persist.tile([P, ET, NQ], f8, tag="XQ")      # my query rows ^T
        WV = persist.tile([P, ET, E], f8, tag="WV")
        WO = persist.tile([P, ET, E], f8, tag="WO")
        VG = [persist.tile([P, 2, VW], f8, tag=f"vg{g}", name=f"vg{g}")
              for g in range(KT // 2)]
        CTG = [persist.tile([P, 2, NQ], f8, tag=f"ctg{t}", name=f"ctg{t}")
               for t in range(ET // 2)]
        if not skip_bias:
            bqs = persist.tile([P, ET], f32, tag="bqs")
            bks = persist.tile([P, ET], f32, tag="bks")
            bvb = persist.tile([P, E], f32, tag="bvb")
        if not skip_affine:
            lngb = persist.tile([P, E], f32, tag="lngb")
            lnbb = persist.tile([P, E], f32, tag="lnbb")
        epsb = persist.tile([P, 1], f32, tag="epsb")

        # ---- input loads, ordered by first use ----
        def load_wslices(q):
            wq_sl = wslice.tile([P, 2, ET, P], f8, tag="wqsl", name="wqsl")
            wk_sl = wslice.tile([P, 2, ET, P], f8, tag="wksl", name="wksl")
            nc.gpsimd.dma_start(out=wq_sl, in_=d_wqR[q])
            nc.gpsimd.dma_start(out=wk_sl, in_=d_wkR[q])
            return wq_sl, wk_sl

        w0 = load_wslices(0)
        if not skip_bias:
            dma.dma_start(out=bqs, in_=d_bq[:].rearrange("(t p) -> p t", p=P))
            dma.dma_start(out=bks, in_=d_bk[:].rearrange("(t p) -> p t", p=P))
        nc.vector.memset(epsb, 1e-5)
        # preload the exp ACT table while DMAs stream
        tdummy = misc.tile([1, 1], f32, tag="tdummy", name="tdummy")
        nc.scalar.activation(out=tdummy, in_=epsb[0:1, 0:1],
                             func=mybir.ActivationFunctionType.Exp)
        # chunked x loads so quad-0 projections start on first chunks
        for ch in range(2):
            csl = slice(ch * 512, (ch + 1) * 512)
            dma.dma_start(out=XQ[:, :, csl],
                          in_=d_xqT[:, csl].rearrange("(t p) k -> p t k", p=P))
        for ch in range(4):
            csl = slice(ch * 512, (ch + 1) * 512)
            dma.dma_start(out=XK[:, :, csl],
                          in_=d_xkT[:, csl].rearrange("(t p) k -> p t k", p=P))
        dma.dma_start(out=WV, in_=d_wvT[:].rearrange("(t p) m -> p t m", p=P))
        if not skip_bias:
            dma.dma_start(out=bvb, in_=bcast_ap(d_bv))
        for g in range(KT // 2):
            v4 = VG[g].rearrange("p j (h w) -> p j h w", w=W65)
            nc.vector.memset(v4[:, :, :, D:W65], 1.0)
        if not skip_affine:
            dma.dma_start(out=lngb, in_=bcast_ap(d_lng))
            dma.dma_start(out=lnbb, in_=bcast_ap(d_lnb))
        dma.dma_start(out=WO, in_=d_woT[:].rearrange("(t p) m -> p t m", p=P))

        # ---- QK projection for one quad (DoubleRow over e-tile pairs) ----
        # emitter order: [Q(pl0,c0), Q(pl1,c0), K(pl0,c0), K(pl1,c0),
        #                 Q(pl0,c1), Q(pl1,c1), K(pl0,c1), K(pl1,c1),
        #                 K(pl0,c2), K(pl1,c2), K(pl0,c3), K(pl1,c3)]
        def qk_emitters(q, wq_sl, wk_sl, qt, kt):
            def eq(pi, ch):
                def em():
                    csl = slice(ch * 512, (ch + 1) * 512)
                    ps = psum.tile([P, 512], f32, tag="acc", name="mmps")
                    for e2 in range(ET // 2):
                        nc.tensor.matmul(
                            ps, wq_sl[:, pi, 2 * e2:2 * e2 + 2, :],
                            XQ[:, 2 * e2:2 * e2 + 2, csl],
                            start=(e2 == 0), stop=(e2 == ET // 2 - 1),
                            perf_mode=DR,
                        )
                    if skip_bias:
                        nc.vector.tensor_copy(qt[:, pi, csl], ps)
                    else:
                        b = 2 * q + pi
                        nc.vector.tensor_scalar_add(
                            out=qt[:, pi, csl], in0=ps, scalar1=bqs[:, b:b + 1])
                return em

            def ek(pi, ch):
                def em():
                    csl = slice(ch * 512, (ch + 1) * 512)
                    ps = psum.tile([P, 512], f32, tag="acc", name="mmps")
                    for e2 in range(ET // 2):
                        nc.tensor.matmul(
                            ps, wk_sl[:, pi, 2 * e2:2 * e2 + 2, :],
                            XK[:, 2 * e2:2 * e2 + 2, csl],
                            start=(e2 == 0), stop=(e2 == ET // 2 - 1),
                            perf_mode=DR,
                        )
                    if skip_bias:
                        nc.vector.tensor_copy(kt[:, pi, csl], ps)
                    else:
                        b = 2 * q + pi
                        nc.vector.tensor_scalar_add(
                            out=kt[:, pi, csl], in0=ps, scalar1=bks[:, b:b + 1])
                return em

            return [eq(0, 0), eq(1, 0), ek(0, 0), ek(1, 0),
                    eq(0, 1), eq(1, 1), ek(0, 1), ek(1, 1),
                    ek(0, 2), ek(1, 2), ek(0, 3), ek(1, 3)]

        def new_qk_tiles():
            qt = qkpool.tile([P, 2, NQ], f8, tag="qtp", name="qtp")
            kt = qkpool.tile([P, 2, S], f8, tag="ktp", name="ktp")
            return qt, kt

        # ---- scores + exp for one group of k-tiles, one head-pair ----
        # unit u = quad*4 + qc*2 + hp; heads 2*hp, 2*hp+1 within the quad
        def scores_exp_group(qt, kt, quad, qc, hp, base, n):
            u = quad * 4 + qc * 2 + hp
            qsl = slice(qc * 512, (qc + 1) * 512)
            sps = [
                psum.tile([P, 3, 512], f32, tag="spsum", name="sps0"),
                psum.tile([P, 3, 512], f32, tag="spsum", name="sps1"),
            ]
            for j in range(n):
                kti = base + j
                ksl = slice(kti * P, (kti + 1) * P)
                for h in range(2):
                    ht = 2 * hp + h
                    hsl = slice(32 * ht, 32 * ht + 32)
                    nc.tensor.matmul(
                        sps[h][:, j, :], kt[hsl, :, ksl], qt[hsl, :, qsl],
                        start=True, stop=True, perf_mode=DR,
                        tile_position=(32 * ht, 0),
                    )
            eng = _exp_engine(u, GROUPS.index((base, n)))
            kpts = []
            for h in range(2):
                pt = ppool.tile([P, 3, 512], f8, tag="pt", name="pt")
                if eng == "A":
                    nc.scalar.activation(
                        out=pt[:, 0:n, :], in_=sps[h][:, 0:n, :],
                        func=mybir.ActivationFunctionType.Exp,
                        scale=0.125,
                    )
                else:
                    nc.vector.tensor_scalar(
                        out=pt[:, 0:n, :].bitcast(u8), in0=sps[h][:, 0:n, :],
                        scalar1=SCHRA_A, scalar2=SCHRA_B,
                        op0=mybir.AluOpType.mult, op1=mybir.AluOpType.add,
                    )
                kpts.append(pt)
            return (kpts[0], kpts[1], base, n)

        def scores_exp(qt, kt, quad, qc, hp, interleave=None):
            pts = []
            for base, n in GROUPS:
                pts.append(scores_exp_group(qt, kt, quad, qc, hp, base, n))
                if interleave:
                    interleave.pop(0)()
            return pts

        # ctx matmuls per group: DoubleRow over even-aligned k-tile pairs
        # (matching the VG pair layout), plain fp8 for the odd leftovers
        def ctx_group_mms(cp, vsl, pt, base, n, first, last_flags):
            segs = []
            if n == 2:
                segs.append((0, 2))
            elif base % 2 == 0:
                segs.append((0, 2)); segs.append((2, 1))
            else:
                segs.append((0, 1)); segs.append((1, 2))
            for i, (j0, w) in enumerate(segs):
                kti = base + j0
                is_last = last_flags and i == len(segs) - 1
                if w == 2:
                    nc.tensor.matmul(
                        cp, VG[kti // 2][:, 0:2, vsl], pt[:, j0:j0 + 2, :],
                        start=first and i == 0, stop=is_last, perf_mode=DR,
                    )
                else:
                    nc.tensor.matmul(
                        cp, VG[kti // 2][:, kti % 2, vsl], pt[:, j0, :],
                        start=first and i == 0, stop=is_last,
                    )

        # ---- ctx + normalize for a previously exp'd unit (2 heads) ----
        def ctx_norm(quad, qc, hp, pts):
            qsl = slice(qc * 512, (qc + 1) * 512)
            cps = [
                psum.tile([65, 512], f32, tag="acc", name="cps0"),
                psum.tile([65, 512], f32, tag="acc", name="cps1"),
            ]
            for gi, (pt0, pt1, base, n) in enumerate(pts):
                for h in range(2):
                    hh = 4 * quad + 2 * hp + h
                    vsl = slice(hh * W65, (hh + 1) * W65)
                    ctx_group_mms(cps[h], vsl, (pt0, pt1)[h], base, n,
                                  first=(gi == 0), last_flags=(gi == len(pts) - 1))
            for h in range(2):
                hh = 4 * quad + 2 * hp + h
                recip = misc.tile([1, 512], f32, tag="recip", name="recip")
                nc.vector.reciprocal(out=recip, in_=cps[h][64:65, :])
                bc = misc.tile([D, 512], f32, tag="bc", name="bc")
                nc.gpsimd.partition_broadcast(out_ap=bc, in_ap=recip)
                r0 = (hh % 2) * D
                # CTG = (cps * 16) * (1/denom) - CTX_SCALE fused into the stt
                nc.vector.scalar_tensor_tensor(
                    out=CTG[hh // 4][r0:r0 + D, (hh // 2) % 2, qsl],
                    in0=cps[h][0:D, :], scalar=CTX_SCALE, in1=bc,
                    op0=mybir.AluOpType.mult, op1=mybir.AluOpType.mult,
                )

        # ---- quad-0 projection with qc0 scores/exp interleaved per K chunk ----
        qk0 = new_qk_tiles()
        ems0 = qk_emitters(0, *w0, *qk0)
        ems0[0]()  # Q plane0 chunk 0
        ems0[1]()  # Q plane1 chunk 0
        pts_u = [[], []]   # hp0, hp1 of (quad0, qc0)
        # after K chunk ch (both planes), k-tiles 0..4ch+3 are ready
        chunk_groups = [[0], [1], [2, 3], [4, 5]]
        kq_order = [2, 3, 4, 5, 6, 7, 8, 9, 10, 11]  # remaining emitter idxs
        ki = 0
        for ch in range(4):
            # K chunk ch, both planes (+ leftover Q chunk-1 before ch1 K)
            take = 4 if ch == 1 else 2
            for _ in range(take):
                ems0[kq_order[ki]](); ki += 1
            for gi in chunk_groups[ch]:
                base, n = GROUPS[gi]
                for hp in range(2):
                    pts_u[hp].append(
                        scores_exp_group(*qk0, 0, 0, hp, base, n))

        # ---- V projection (DoubleRow); quad-0 qc1 exps and quad-1 projection
        # both interleave under it so ACT/DVE never starve at the handoff
        w1 = load_wslices(1)
        qk1 = new_qk_tiles()
        ems1 = qk_emitters(1, *w1, *qk1)
        pts_qc1 = [[], []]  # hp0, hp1 of (quad0, qc1)
        # 12 groups over 8 V steps; quad-1 emitters trail by one step
        vg_groups = [2, 2, 2, 2, 1, 1, 1, 1]
        vg_ems = [0, 2, 2, 2, 2, 2, 1, 1]
        gidx = 0
        for g in range(KT // 2):
            for j in range(2):
                kti = 2 * g + j
                ksl = slice(kti * P, (kti + 1) * P)
                for ch in range(2):
                    csl = slice(ch * 512, (ch + 1) * 512)
                    ps = psum.tile([P, 512], f32, tag="acc", name="mmps")
                    for e2 in range(ET // 2):
                        nc.tensor.matmul(
                            ps, XK[:, 2 * e2:2 * e2 + 2, ksl],
                            WV[:, 2 * e2:2 * e2 + 2, csl],
                            start=(e2 == 0), stop=(e2 == ET // 2 - 1),
                            perf_mode=DR,
                        )
                    v4 = VG[g].rearrange("p j (h w) -> p j h w", w=W65)
                    if skip_bias:
                        nc.vector.tensor_copy(
                            v4[:, j, ch * 8:(ch + 1) * 8, 0:D],
                            ps.rearrange("p (h w) -> p h w", w=D))
                    else:
                        nc.vector.tensor_add(
                            out=v4[:, j, ch * 8:(ch + 1) * 8, 0:D],
                            in0=ps.rearrange("p (h w) -> p h w", w=D),
                            in1=bvb[:, csl].rearrange("p (h w) -> p h w", w=D),
                        )
            for _ in range(vg_groups[g]):
                hp, gi = divmod(gidx, 6)
                base, n = GROUPS[gi]
                pts_qc1[hp].append(
                    scores_exp_group(*qk0, 0, 1, hp, base, n))
                gidx += 1
            for _ in range(vg_ems[g]):
                if ems1:
                    ems1.pop(0)()

        # ---- quad-0 qc0 ctx, remaining quad-1 projection ----
        ctx_norm(0, 0, 0, pts_u[0])
        ctx_norm(0, 0, 1, pts_u[1])
        for em in ems1:
            em()

        # ---- pipelined attention: scores/exp unit i overlaps ctx unit i-2 ----
        seq = [(q, qc, hp) for q in range(1, NQUAD)
               for qc in range(2) for hp in range(2)]
        cur_qk = qk1
        nxt_qk = None
        wnxt = None
        pending = []
        prevq = [(0, 1, 0, pts_qc1[0]), (0, 1, 1, pts_qc1[1])]
        for q, qc, hp in seq:
            if (qc, hp) == (0, 0) and q > 1:
                cur_qk = nxt_qk
            if (qc, hp) == (1, 0) and q + 1 < NQUAD:
                # stage next quad's projection: DMA now, matmuls interleaved
                wnxt = load_wslices(q + 1)
                nxt_qk = new_qk_tiles()
                pending = qk_emitters(q + 1, *wnxt, *nxt_qk)
            pts = scores_exp(*cur_qk, q, qc, hp, interleave=pending)
            pq, pqc, php, ppts = prevq.pop(0)
            ctx_norm(pq, pqc, php, ppts)
            prevq.append((q, qc, hp, pts))
            if (qc, hp) == (1, 1):
                for em in pending:
                    em()
                pending = []

        # ---- out projection (DoubleRow) + residual + layernorm ----
        # residual DMA (d_xq already carries +bo, folded on host); LN stats via
        # fused accum_out sums (mean) + an ACT Square pass (E[x^2]) - no
        # bn_stats pass, and the (x-mu)*rstd normalize runs on the idle ACT.
        xq_sbs = []
        for nt in range(ET):
            xq_sb = xqp.tile([P, E], f32, tag="xqt", name="xqt")
            dma.dma_start(out=xq_sb, in_=d_xq[nt * P:(nt + 1) * P, :])
            xq_sbs.append(xq_sb)

        def tail_nt(nt):
            nsl = slice(nt * P, (nt + 1) * P)
            xq_sb = xq_sbs[nt]
            out_sb = outp.tile([P, E], f32, tag="outsb", name="outsb")
            rsum = misc.tile([P, 2], f32, tag="rsum", name="rsum")
            scratch = outp.tile([P, E], f32, tag="sq", name="sq")
            sq2 = misc.tile([P, 1], f32, tag="sq2", name="sq2")
            for ec in range(2):
                csl = slice(ec * 512, (ec + 1) * 512)
                ps = psum.tile([P, 512], f32, tag="acc", name="mmps")
                for t in range(ET // 2):
                    nc.tensor.matmul(
                        ps, CTG[t][:, 0:2, nsl], WO[:, 2 * t:2 * t + 2, csl],
                        start=(t == 0), stop=(t == ET // 2 - 1),
                        perf_mode=DR,
                    )
                # out = ps/CTX_SCALE + (x + bo), accumulating row sums
                nc.vector.scalar_tensor_tensor(
                    out=out_sb[:, csl], in0=ps, scalar=1.0 / CTX_SCALE,
                    in1=xq_sb[:, csl],
                    op0=mybir.AluOpType.mult, op1=mybir.AluOpType.add,
                    accum_out=rsum[:, ec:ec + 1],
                )
            nc.scalar.activation(
                out=scratch, in_=out_sb,
                func=mybir.ActivationFunctionType.Square,
                accum_out=sq2)
            # var*E = sq2 - (r0+r1)^2/E; the final /E folds into Sqrt's scale
            stat = misc.tile([P, 4], f32, tag="stat", name="stat")
            nc.vector.tensor_add(out=stat[:, 0:1], in0=rsum[:, 0:1], in1=rsum[:, 1:2])
            nc.vector.scalar_tensor_tensor(
                out=stat[:, 1:2], in0=stat[:, 0:1], scalar=1.0 / E,
                in1=stat[:, 0:1],
                op0=mybir.AluOpType.mult, op1=mybir.AluOpType.mult)  # msum^2/E
            nc.vector.tensor_sub(out=stat[:, 3:4], in0=sq2, in1=stat[:, 1:2])
            std = misc.tile([P, 1], f32, tag="std", name="std")
            nc.scalar.activation(
                out=std, in_=stat[:, 3:4],
                func=mybir.ActivationFunctionType.Sqrt,
                bias=epsb[:, 0:1], scale=1.0 / E,
            )
            nc.vector.reciprocal(out=std, in_=std)                # rstd
            negmr = misc.tile([P, 1], f32, tag="negmr", name="negmr")
            nc.vector.scalar_tensor_tensor(
                out=negmr, in0=stat[:, 0:1], scalar=-1.0 / E, in1=std,
                op0=mybir.AluOpType.mult, op1=mybir.AluOpType.mult)  # -mean*rstd
            # (x - mu) * rstd == x*rstd + (-mu*rstd), on ACT
            nc.scalar.activation(
                out=out_sb, in_=out_sb,
                func=mybir.ActivationFunctionType.Identity,
                scale=std[:, 0:1], bias=negmr[:, 0:1],
            )
            if not skip_affine:
                nc.vector.tensor_mul(out=out_sb, in0=out_sb, in1=lngb)
                nc.gpsimd.tensor_add(out=out_sb, in0=out_sb, in1=lnbb)
            dma.dma_start(out=d_out[nsl, :], in_=out_sb)

        pq, pqc, php, ppts = prevq.pop(0)
        ctx_norm(pq, pqc, php, ppts)
        pq, pqc, php, ppts = prevq.pop(0)
        ctx_norm(pq, pqc, php, ppts)
        for nt in range(ET):
            tail_nt(nt)

    nc.compile()
    return nc


def _get_nc(skip_affine=False, skip_bias=False):
    key = ("nc", skip_affine, skip_bias)
    if key not in _cache:
        _cache[key] = _build_nc(skip_affine, skip_bias)
    return _cache[key]


def kernel(x, Wq, bq, Wk, bk, Wv, bv, Wo, bo, ln_g, ln_b, _trace=False, _tmpdir=None):
    from concourse.bass_utils import run_bass_kernel_spmd

    x = np.asarray(x, np.float32)

    # e_out permutation for the quad-DR scores layout
    m = np.arange(P)
    eo = np.empty((NQUAD, 2, P), np.int64)
    for q in range(NQUAD):
        for pi in range(2):
            eo[q, pi] = 64 * (4 * q + m // 32) + 32 * pi + (m % 32)

    def shuffle_w_qk(W):
        # W.T [e_in, e_out] -> [quad, p, plane, t, m]; e_in = t*128 + p
        wT = np.asarray(W, np.float32).T.reshape(ET, P, E)  # [t, p, e_out]
        out = np.empty((NQUAD, P, 2, ET, P), np.float32)
        for q in range(NQUAD):
            for pi in range(2):
                out[q, :, pi, :, :] = wT[:, :, eo[q, pi]].transpose(1, 0, 2)
        return np.ascontiguousarray(out).astype(FP8)

    def perm_bias(b):
        b = np.asarray(b, np.float32)
        # bqs tile loads "(t p) -> p t": vec[blk*128 + m] = b[e_out(blk, m)]
        vec = np.empty(E, np.float32)
        for q in range(NQUAD):
            for pi in range(2):
                vec[(2 * q + pi) * P + m] = b[eo[q, pi]]
        return vec

    wqR = shuffle_w_qk(Wq)
    wkR = shuffle_w_qk(Wk)
    wvT = np.ascontiguousarray(np.asarray(Wv, np.float32).T).astype(FP8)
    woT = np.ascontiguousarray(np.asarray(Wo, np.float32).T).astype(FP8)
    vecs = {
        "bq": perm_bias(bq), "bk": perm_bias(bk),
        "bv": np.asarray(bv, np.float32),
        "lng": np.asarray(ln_g, np.float32), "lnb": np.asarray(ln_b, np.float32),
    }
    bo_f = np.asarray(bo, np.float32)

    in_maps = []
    for c in range(NCORES):
        b, half = c // 2, c % 2
        xbT = np.ascontiguousarray(x[b].T).astype(FP8)
        in_maps.append({
            "xkT": xbT,
            "xqT": np.ascontiguousarray(xbT[:, half * NQ:(half + 1) * NQ]),
            # residual rows with the out-proj bias pre-added (saves a DVE pass)
            "xq": x[b, half * NQ:(half + 1) * NQ, :] + bo_f,
            "wqR": wqR, "wkR": wkR, "wvT": wvT, "woT": woT,
            **vecs,
        })

    # ln_g == 1 / ln_b == 0 make the LN affine step an exact no-op; build the
    # specialized kernel for that case (general path kept as fallback)
    skip_affine = bool(
        np.all(np.asarray(ln_g) == 1.0) and np.all(np.asarray(ln_b) == 0.0))
    skip_bias = bool(
        np.all(np.asarray(bq) == 0.0) and np.all(np.asarray(bk) == 0.0)
        and np.all(np.asarray(bv) == 0.0))
    if skip_bias:
        for mm in in_maps:
            del mm["bq"], mm["bk"], mm["bv"]
    nc = _get_nc(skip_affine, skip_bias)
    _cache["last_nc"] = nc
    res = run_bass_kernel_spmd(
        nc, in_maps, list(range(NCORES)), trace=_trace, tmpdir=_tmpdir
    )
    out = np.empty((B, S, E), np.float32)
    for c in range(NCORES):
        b, half = c // 2, c % 2
        out[b, half * NQ:(half + 1) * NQ, :] = res.results[c]["out"]
    if _trace:
        _cache["last_result"] = res
    return out


# revision 11
# speedup vs baseline: 1.1461x; 1.1461x over previous
"""Trainium2 Bass kernel for a dense transformer attention block.

Reference (per batch b of 4, seq S=2048, embed E=1024, H=16 heads, D=64):
    q/k/v = x @ W{q,k,v}.T + b,  split heads
    attn  = softmax(q k^T / sqrt(D)),  ctx = attn @ v
    out   = LN(ctx @ Wo.T + bo + x) * ln_g + ln_b

Sharding (8 cores, no collectives): core c handles batch b=c//2 and query rows
[1024*(c%2), 1024*(c%2+1)).  Each core computes K/V projections for its full
batch (duplicated with its pair core, ~25% extra FLOPs, zero comms), attention
for all 16 heads over its 1024 query rows, out-projection + residual + LN for
its rows.  Host reassembles the 8 row-shards.

Core layout strategy:
  - scores computed TRANSPOSED: S^T[k, q] with K^T stationary / Q^T moving, so
    exp(S^T) feeds the ctx matmul directly as the moving operand - no PE
    transposes anywhere in the kernel.
  - QUAD-head q/k layout: 4 heads per 128 partitions, head-dim split as
    d = 32*plane + (p%32) with the plane in a free dim.  Scores then run as
    fp8 DoubleRow matmuls (Ki=32, Ko=2) at 0.5 cyc/row - 2x the throughput of
    the plain-fp8 two-head row-group scheme.
  - exp split between ACT (true exp) and DVE (Schraudolph: fp8 bit-trick
    u8 = rne(score*log2(e) + 56) bitcast as fp8e4m3 ~= exp(score/8)); both
    paths share scale so they mix freely within one softmax row.  The split
    ratio balances the two engines, which are otherwise the bottleneck.
  - softmax denominator from a ones-column appended to V (stationary [V_h|1],
    M=65): PSUM row 64 accumulates sum_k exp.  Normalization (x16 to keep fp8
    ctx out of the subnormal range; /16 folded into the out-proj epilogue) is
    fused into the PSUM->SBUF cast via partition_broadcast of the reciprocal.
  - ctx lands as ctx^T[e, q], exactly the stationary layout out-proj needs.
  - q/k/v/out projections and ctx run in fp8e4 with DoubleRow (0.5 cyc/row);
    PSUM accumulation is always fp32, residual + layernorm in fp32.  The
    attention branch contributes only ~0.8% of the output magnitude (the
    residual dominates), so fp8 path error dilutes ~128x in the final output.
  - scores+exp for unit i overlap ctx for unit i-1 (software pipeline), and
    the next quad's projection matmuls are interleaved into the attention
    stream so the in-order PE queue always has work.
"""

import sys

if "/opt/trn_rl_repo" not in sys.path:
    sys.path.insert(0, "/opt/trn_rl_repo")

import numpy as np
import ml_dtypes

B, S, E = 4, 2048, 1024
H, D = 16, 64
NQ = S // 2          # query rows per core
P = 128
ET = E // P          # 8 e-tiles
KT = S // P          # 16 k-tiles
W65 = D + 1          # V head width incl. ones column
VW = H * W65         # 1040
NCORES = 8
NQUAD = 4            # head quads (4 heads on 128 partitions)
CTX_SCALE = 16.0     # keep fp8 ctx in normal range

SCHRA_A = 1.4426950408889634   # log2(e): u8 = rne(A*s + B), bits = fp8e4m3
SCHRA_B = 56.0                 # exponent bias 7 * 8 mantissa steps

FP8 = ml_dtypes.float8_e4m3

_cache = {}

# k-tile grouping for exp batching: 3-bank PSUM groups amortize ACT/DVE
# per-instruction overhead over up to 1536 columns
GROUPS = [(0, 3), (3, 3), (6, 3), (9, 3), (12, 2), (14, 2)]


def _exp_engine(u, g, h):
    """Engine for the exp of unit u (0..15), group g (0..5), head h (0/1):
    ACT (true exp, 0.83 ns/el) or DVE (Schraudolph, 1.04 ns/el).  Head-level
    split keeps BOTH engines fed from every group (the two heads sit in
    separate PSUM slots, so the two exp pipelines run independently)."""
    if g >= 4:
        return "A" if u % 2 == 0 else ("A" if h == 0 else "D")
    return "A" if h == 0 else "D"


def _build_nc(skip_affine=False, skip_bias=False):
    import concourse.bass as bass
    import concourse.tile as tile
    from concourse import bacc, mybir

    f8 = mybir.dt.float8e4
    f32 = mybir.dt.float32
    u8 = mybir.dt.uint8
    DR = mybir.MatmulPerfMode.DoubleRow

    nc = bacc.Bacc(None, target_bir_lowering=False, debug=False)

    d_xkT = nc.dram_tensor("xkT", [E, S], f8, kind="ExternalInput")
    d_xqT = nc.dram_tensor("xqT", [E, NQ], f8, kind="ExternalInput")
    d_xq = nc.dram_tensor("xq", [NQ, E], f32, kind="ExternalInput")
    # wq/wk pre-shuffled on host to [quad, p, plane, t, m] so each quad's
    # slice DMAs as contiguous 2KB runs; e_out(quad, plane, m) =
    # 64*(4*quad + m//32) + 32*plane + m%32 (the quad-DR scores layout)
    d_wqR = nc.dram_tensor("wqR", [NQUAD, P, 2, ET, P], f8, kind="ExternalInput")
    d_wkR = nc.dram_tensor("wkR", [NQUAD, P, 2, ET, P], f8, kind="ExternalInput")
    d_wvT = nc.dram_tensor("wvT", [E, E], f8, kind="ExternalInput")
    d_woT = nc.dram_tensor("woT", [E, E], f8, kind="ExternalInput")
    if not skip_bias:
        d_bq = nc.dram_tensor("bq", [E], f32, kind="ExternalInput")
        d_bk = nc.dram_tensor("bk", [E], f32, kind="ExternalInput")
        d_bv = nc.dram_tensor("bv", [E], f32, kind="ExternalInput")
    d_lng = nc.dram_tensor("lng", [E], f32, kind="ExternalInput")
    d_lnb = nc.dram_tensor("lnb", [E], f32, kind="ExternalInput")
    d_out = nc.dram_tensor("out", [NQ, E], f32, kind="ExternalOutput")

    def bcast_ap(d):
        ap = d[:]
        return bass.AP(tensor=ap.tensor, offset=ap.offset, ap=[[0, P], [1, E]])

    from contextlib import ExitStack

    with tile.TileContext(nc) as tc, ExitStack() as ctx:
        persist = ctx.enter_context(tc.tile_pool(name="persist", bufs=1))
        wslice = ctx.enter_context(tc.tile_pool(name="wslice", bufs=2))
        qkpool = ctx.enter_context(tc.tile_pool(name="qkpool", bufs=2))
        ppool = ctx.enter_context(tc.tile_pool(name="ppool", bufs=26))
        misc = ctx.enter_context(tc.tile_pool(name="misc", bufs=4))
        xqp_bufs = 6 if (skip_affine and skip_bias) else 4
        xqp = ctx.enter_context(tc.tile_pool(name="xqp", bufs=xqp_bufs))
        outp_bufs = 4 if (skip_affine and skip_bias) else 3
        outp = ctx.enter_context(tc.tile_pool(name="outp", bufs=outp_bufs))
        cpool = ctx.enter_context(tc.tile_pool(name="cpool", bufs=4))
        psum = ctx.enter_context(tc.tile_pool(name="psum", bufs=2, space="PSUM"))

        dma = nc.sync

        # ---- persistent tiles ----
        XK = persist.tile([P, ET, S], f8, tag="XK")       # x[b]^T, e-tiles on dim1
        XQ = persist.tile([P, ET, NQ], f8, tag="XQ")      # my query rows ^T
        WV = persist.tile([P, ET, E], f8, tag="WV")
        WO = persist.tile([P, ET, E], f8, tag="WO")
        VG = [persist.tile([P, 2, VW], f8, tag=f"vg{g}", name=f"vg{g}")
              for g in range(KT // 2)]
        CTG = [persist.tile([P, 2, NQ], f8, tag=f"ctg{t}", name=f"ctg{t}")
               for t in range(ET // 2)]
        if not skip_bias:
            bqs = persist.tile([P, ET], f32, tag="bqs")
            bks = persist.tile([P, ET], f32, tag="bks")
            bvb = persist.tile([P, E], f32, tag="bvb")
        if not skip_affine:
            lngb = persist.tile([P, E], f32, tag="lngb")
            lnbb = persist.tile([P, E], f32, tag="lnbb")
        epsb = persist.tile([P, 1], f32, tag="epsb")

        # ---- input loads, ordered by first use ----
        def load_wslices(q):
            wq_sl = wslice.tile([P, 2, ET, P], f8, tag="wqsl", name="wqsl")
            wk_sl = wslice.tile([P, 2, ET, P], f8, tag="wksl", name="wksl")
            nc.gpsimd.dma_start(out=wq_sl, in_=d_wqR[q])
            nc.gpsimd.dma_start(out=wk_sl, in_=d_wkR[q])
            return wq_sl, wk_sl

        w0 = load_wslices(0)
        if not skip_bias:
            dma.dma_start(out=bqs, in_=d_bq[:].rearrange("(t p) -> p t", p=P))
            dma.dma_start(out=bks, in_=d_bk[:].rearrange("(t p) -> p t", p=P))
        nc.vector.memset(epsb, 1e-5)
        # preload the exp ACT table while DMAs stream
        tdummy = misc.tile([1, 1], f32, tag="tdummy", name="tdummy")
        nc.scalar.activation(out=tdummy, in_=epsb[0:1, 0:1],
                             func=mybir.ActivationFunctionType.Exp)
        # chunked x loads so quad-0 projections start on first chunks
        for ch in range(2):
            csl = slice(ch * 512, (ch + 1) * 512)
            dma.dma_start(out=XQ[:, :, csl],
                          in_=d_xqT[:, csl].rearrange("(t p) k -> p t k", p=P))
        for ch in range(4):
            csl = slice(ch * 512, (ch + 1) * 512)
            dma.dma_start(out=XK[:, :, csl],
                          in_=d_xkT[:, csl].rearrange("(t p) k -> p t k", p=P))
        dma.dma_start(out=WV, in_=d_wvT[:].rearrange("(t p) m -> p t m", p=P))
        if not skip_bias:
            dma.dma_start(out=bvb, in_=bcast_ap(d_bv))
        for g in range(KT // 2):
            v4 = VG[g].rearrange("p j (h w) -> p j h w", w=W65)
            nc.vector.memset(v4[:, :, :, D:W65], 1.0)
        if not skip_affine:
            dma.dma_start(out=lngb, in_=bcast_ap(d_lng))
            dma.dma_start(out=lnbb, in_=bcast_ap(d_lnb))
        dma.dma_start(out=WO, in_=d_woT[:].rearrange("(t p) m -> p t m", p=P))

        # ---- QK projection for one quad (DoubleRow over e-tile pairs) ----
        # emitter order: [Q(pl0,c0), Q(pl1,c0), K(pl0,c0), K(pl1,c0),
        #                 Q(pl0,c1), Q(pl1,c1), K(pl0,c1), K(pl1,c1),
        #                 K(pl0,c2), K(pl1,c2), K(pl0,c3), K(pl1,c3)]
        # cast_eng: "A" puts the PSUM->SBUF cast on ACT (used for quad 0,
        # when ACT has no exp work yet), "D" on DVE.
        def cast_to(eng, out, ps, bias):
            if eng == "A":
                nc.scalar.activation(
                    out=out, in_=ps,
                    func=mybir.ActivationFunctionType.Identity,
                    bias=0.0 if bias is None else bias)
            elif bias is None:
                nc.vector.tensor_copy(out, ps)
            else:
                nc.vector.tensor_scalar_add(out=out, in0=ps, scalar1=bias)

        def qk_emitters(q, wq_sl, wk_sl, qt, kt, cast_eng="D"):
            def eq(pi, ch):
                def em():
                    csl = slice(ch * 512, (ch + 1) * 512)
                    ps = psum.tile([P, 512], f32, tag="acc", name="mmps")
                    for e2 in range(ET // 2):
                        nc.tensor.matmul(
                            ps, wq_sl[:, pi, 2 * e2:2 * e2 + 2, :],
                            XQ[:, 2 * e2:2 * e2 + 2, csl],
                            start=(e2 == 0), stop=(e2 == ET // 2 - 1),
                            perf_mode=DR,
                        )
                    b = 2 * q + pi
                    cast_to(cast_eng, qt[:, pi, csl], ps,
                            None if skip_bias else bqs[:, b:b + 1])
                return em

            def ek(pi, ch):
                def em():
                    csl = slice(ch * 512, (ch + 1) * 512)
                    ps = psum.tile([P, 512], f32, tag="acc", name="mmps")
                    for e2 in range(ET // 2):
                        nc.tensor.matmul(
                            ps, wk_sl[:, pi, 2 * e2:2 * e2 + 2, :],
                            XK[:, 2 * e2:2 * e2 + 2, csl],
                            start=(e2 == 0), stop=(e2 == ET // 2 - 1),
                            perf_mode=DR,
                        )
                    b = 2 * q + pi
                    cast_to(cast_eng, kt[:, pi, csl], ps,
                            None if skip_bias else bks[:, b:b + 1])
                return em

            return [eq(0, 0), eq(1, 0), ek(0, 0), ek(1, 0),
                    eq(0, 1), eq(1, 1), ek(0, 1), ek(1, 1),
                    ek(0, 2), ek(1, 2), ek(0, 3), ek(1, 3)]

        def new_qk_tiles():
            qt = qkpool.tile([P, 2, NQ], f8, tag="qtp", name="qtp")
            kt = qkpool.tile([P, 2, S], f8, tag="ktp", name="ktp")
            return qt, kt

        # ---- scores + exp for one group of k-tiles, one head-pair ----
        # unit u = quad*4 + qc*2 + hp; heads 2*hp, 2*hp+1 within the quad
        def scores_exp_group(qt, kt, quad, qc, hp, base, n):
            u = quad * 4 + qc * 2 + hp
            qsl = slice(qc * 512, (qc + 1) * 512)
            sps = [
                psum.tile([P, 3, 512], f32, tag="spsum", name="sps0"),
                psum.tile([P, 3, 512], f32, tag="spsum", name="sps1"),
            ]
            for j in range(n):
                kti = base + j
                ksl = slice(kti * P, (kti + 1) * P)
                for h in range(2):
                    ht = 2 * hp + h
                    hsl = slice(32 * ht, 32 * ht + 32)
                    nc.tensor.matmul(
                        sps[h][:, j, :], kt[hsl, :, ksl], qt[hsl, :, qsl],
                        start=True, stop=True, perf_mode=DR,
                        tile_position=(32 * ht, 0),
                    )
            gi = GROUPS.index((base, n))
            kpts = []
            for h in range(2):
                eng = _exp_engine(u, gi, h)
                pt = ppool.tile([P, 3, 512], f8, tag="pt", name="pt")
                if eng == "A":
                    nc.scalar.activation(
                        out=pt[:, 0:n, :], in_=sps[h][:, 0:n, :],
                        func=mybir.ActivationFunctionType.Exp,
                        scale=0.125,
                    )
                else:
                    nc.vector.tensor_scalar(
                        out=pt[:, 0:n, :].bitcast(u8), in0=sps[h][:, 0:n, :],
                        scalar1=SCHRA_A, scalar2=SCHRA_B,
                        op0=mybir.AluOpType.mult, op1=mybir.AluOpType.add,
                    )
                kpts.append(pt)
            return (kpts[0], kpts[1], base, n)

        def scores_exp(qt, kt, quad, qc, hp, interleave=None):
            pts = []
            for base, n in GROUPS:
                pts.append(scores_exp_group(qt, kt, quad, qc, hp, base, n))
                if interleave:
                    interleave.pop(0)()
            return pts

        # ctx matmuls per group: DoubleRow over even-aligned k-tile pairs
        # (matching the VG pair layout), plain fp8 for the odd leftovers
        def ctx_group_mms(cp, vsl, pt, base, n, first, last_flags):
            segs = []
            if n == 2:
                segs.append((0, 2))
            elif base % 2 == 0:
                segs.append((0, 2)); segs.append((2, 1))
            else:
                segs.append((0, 1)); segs.append((1, 2))
            for i, (j0, w) in enumerate(segs):
                kti = base + j0
                is_last = last_flags and i == len(segs) - 1
                if w == 2:
                    nc.tensor.matmul(
                        cp, VG[kti // 2][:, 0:2, vsl], pt[:, j0:j0 + 2, :],
                        start=first and i == 0, stop=is_last, perf_mode=DR,
                    )
                else:
                    nc.tensor.matmul(
                        cp, VG[kti // 2][:, kti % 2, vsl], pt[:, j0, :],
                        start=first and i == 0, stop=is_last,
                    )

        # ---- ctx + normalize for a previously exp'd unit (2 heads) ----
        # CTX_SCALE is folded into wvT on the host (ones column unaffected),
        # so the normalize is a plain multiply: one ACT copy evacuates the
        # whole ctx+denominator PSUM block, DVE reciprocals the denominator
        # row, and the broadcast + multiply run on the otherwise-idle Pool.
        def ctx_norm(quad, qc, hp, pts):
            qsl = slice(qc * 512, (qc + 1) * 512)
            cps = [
                psum.tile([65, 512], f32, tag="acc", name="cps0"),
                psum.tile([65, 512], f32, tag="acc", name="cps1"),
            ]
            for gi, (pt0, pt1, base, n) in enumerate(pts):
                for h in range(2):
                    hh = 4 * quad + 2 * hp + h
                    vsl = slice(hh * W65, (hh + 1) * W65)
                    ctx_group_mms(cps[h], vsl, (pt0, pt1)[h], base, n,
                                  first=(gi == 0), last_flags=(gi == len(pts) - 1))
            for h in range(2):
                hh = 4 * quad + 2 * hp + h
                cs = cpool.tile([65, 512], f32, tag="cs", name="cs")
                nc.scalar.activation(
                    out=cs, in_=cps[h],
                    func=mybir.ActivationFunctionType.Identity)
                recip = misc.tile([1, 512], f32, tag="recip", name="recip")
                nc.vector.reciprocal(out=recip, in_=cs[64:65, :])
                bc = misc.tile([D, 512], f32, tag="bc", name="bc")
                nc.gpsimd.partition_broadcast(out_ap=bc, in_ap=recip)
                r0 = (hh % 2) * D
                nc.gpsimd.tensor_tensor(
                    out=CTG[hh // 4][r0:r0 + D, (hh // 2) % 2, qsl],
                    in0=cs[0:D, :], in1=bc, op=mybir.AluOpType.mult,
                )

        # ---- quad-0 projection with qc0 scores/exp interleaved per K chunk ----
        qk0 = new_qk_tiles()
        ems0 = qk_emitters(0, *w0, *qk0, cast_eng="A")
        ems0[0]()  # Q plane0 chunk 0
        ems0[1]()  # Q plane1 chunk 0
        pts_u = [[], []]   # hp0, hp1 of (quad0, qc0)
        # after K chunk ch (both planes), k-tiles 0..4ch+3 are ready
        chunk_groups = [[0], [1], [2, 3], [4, 5]]
        kq_order = [2, 3, 4, 5, 6, 7, 8, 9, 10, 11]  # remaining emitter idxs
        ki = 0
        for ch in range(4):
            # K chunk ch, both planes (+ leftover Q chunk-1 before ch1 K)
            take = 4 if ch == 1 else 2
            for _ in range(take):
                ems0[kq_order[ki]](); ki += 1
            for gi in chunk_groups[ch]:
                base, n = GROUPS[gi]
                for hp in range(2):
                    pts_u[hp].append(
                        scores_exp_group(*qk0, 0, 0, hp, base, n))

        # ---- V projection (DoubleRow); quad-0 qc1 exps and quad-1 projection
        # both interleave under it so ACT/DVE never starve at the handoff
        w1 = load_wslices(1)
        qk1 = new_qk_tiles()
        ems1 = qk_emitters(1, *w1, *qk1)
        pts_qc1 = [[], []]  # hp0, hp1 of (quad0, qc1)
        # 12 groups over 8 V steps; quad-1 emitters trail by one step
        vg_groups = [2, 2, 2, 2, 1, 1, 1, 1]
        vg_ems = [0, 2, 2, 2, 2, 2, 1, 1]
        gidx = 0
        for g in range(KT // 2):
            for j in range(2):
                kti = 2 * g + j
                ksl = slice(kti * P, (kti + 1) * P)
                for ch in range(2):
                    csl = slice(ch * 512, (ch + 1) * 512)
                    ps = psum.tile([P, 512], f32, tag="acc", name="mmps")
                    for e2 in range(ET // 2):
                        nc.tensor.matmul(
                            ps, XK[:, 2 * e2:2 * e2 + 2, ksl],
                            WV[:, 2 * e2:2 * e2 + 2, csl],
                            start=(e2 == 0), stop=(e2 == ET // 2 - 1),
                            perf_mode=DR,
                        )
                    v4 = VG[g].rearrange("p j (h w) -> p j h w", w=W65)
                    if skip_bias:
                        nc.vector.tensor_copy(
                            v4[:, j, ch * 8:(ch + 1) * 8, 0:D],
                            ps.rearrange("p (h w) -> p h w", w=D))
                    else:
                        nc.vector.tensor_add(
                            out=v4[:, j, ch * 8:(ch + 1) * 8, 0:D],
                            in0=ps.rearrange("p (h w) -> p h w", w=D),
                            in1=bvb[:, csl].rearrange("p (h w) -> p h w", w=D),
                        )
            for _ in range(vg_groups[g]):
                hp, gi = divmod(gidx, 6)
                base, n = GROUPS[gi]
                pts_qc1[hp].append(
                    scores_exp_group(*qk0, 0, 1, hp, base, n))
                gidx += 1
            for _ in range(vg_ems[g]):
                if ems1:
                    ems1.pop(0)()

        # ---- quad-0 qc0 ctx, remaining quad-1 projection ----
        ctx_norm(0, 0, 0, pts_u[0])
        ctx_norm(0, 0, 1, pts_u[1])
        for em in ems1:
            em()

        # ---- pipelined attention: scores/exp unit i overlaps ctx unit i-2 ----
        seq = [(q, qc, hp) for q in range(1, NQUAD)
               for qc in range(2) for hp in range(2)]
        cur_qk = qk1
        nxt_qk = None
        wnxt = None
        pending = []
        prevq = [(0, 1, 0, pts_qc1[0]), (0, 1, 1, pts_qc1[1])]
        for q, qc, hp in seq:
            if (qc, hp) == (0, 0) and q > 1:
                cur_qk = nxt_qk
            if (qc, hp) == (1, 0) and q + 1 < NQUAD:
                # stage next quad's projection: DMA now, matmuls interleaved
                wnxt = load_wslices(q + 1)
                nxt_qk = new_qk_tiles()
                pending = qk_emitters(q + 1, *wnxt, *nxt_qk)
            pts = scores_exp(*cur_qk, q, qc, hp, interleave=pending)
            pq, pqc, php, ppts = prevq.pop(0)
            ctx_norm(pq, pqc, php, ppts)
            prevq.append((q, qc, hp, pts))
            if (qc, hp) == (1, 1):
                for em in pending:
                    em()
                pending = []

        # ---- out projection (DoubleRow) + residual + layernorm ----
        # residual DMA (d_xq already carries +bo, folded on host); LN stats via
        # fused accum_out sums (mean) + an ACT Square pass (E[x^2]) - no
        # bn_stats pass, and the (x-mu)*rstd normalize runs on the idle ACT.
        xq_sbs = []
        for nt in range(ET):
            xq_sb = xqp.tile([P, E], f32, tag="xqt", name="xqt")
            dma.dma_start(out=xq_sb, in_=d_xq[nt * P:(nt + 1) * P, :])
            xq_sbs.append(xq_sb)

        def tail_nt(nt):
            nsl = slice(nt * P, (nt + 1) * P)
            xq_sb = xq_sbs[nt]
            out_sb = outp.tile([P, E], f32, tag="outsb", name="outsb")
            rsum = misc.tile([P, 2], f32, tag="rsum", name="rsum")
            scratch = outp.tile([P, E], f32, tag="sq", name="sq")
            sq2 = misc.tile([P, 1], f32, tag="sq2", name="sq2")
            for ec in range(2):
                csl = slice(ec * 512, (ec + 1) * 512)
                ps = psum.tile([P, 512], f32, tag="acc", name="mmps")
                for t in range(ET // 2):
                    nc.tensor.matmul(
                        ps, CTG[t][:, 0:2, nsl], WO[:, 2 * t:2 * t + 2, csl],
                        start=(t == 0), stop=(t == ET // 2 - 1),
                        perf_mode=DR,
                    )
                # out = ps/CTX_SCALE + (x + bo), accumulating row sums
                nc.vector.scalar_tensor_tensor(
                    out=out_sb[:, csl], in0=ps, scalar=1.0 / CTX_SCALE,
                    in1=xq_sb[:, csl],
                    op0=mybir.AluOpType.mult, op1=mybir.AluOpType.add,
                    accum_out=rsum[:, ec:ec + 1],
                )
            nc.scalar.activation(
                out=scratch, in_=out_sb,
                func=mybir.ActivationFunctionType.Square,
                accum_out=sq2)
            # var*E = sq2 - (r0+r1)^2/E; the final /E folds into Sqrt's scale
            stat = misc.tile([P, 4], f32, tag="stat", name="stat")
            nc.vector.tensor_add(out=stat[:, 0:1], in0=rsum[:, 0:1], in1=rsum[:, 1:2])
            nc.vector.scalar_tensor_tensor(
                out=stat[:, 1:2], in0=stat[:, 0:1], scalar=1.0 / E,
                in1=stat[:, 0:1],
                op0=mybir.AluOpType.mult, op1=mybir.AluOpType.mult)  # msum^2/E
            nc.vector.tensor_sub(out=stat[:, 3:4], in0=sq2, in1=stat[:, 1:2])
            std = misc.tile([P, 1], f32, tag="std", name="std")
            nc.scalar.activation(
                out=std, in_=stat[:, 3:4],
                func=mybir.ActivationFunctionType.Sqrt,
                bias=epsb[:, 0:1], scale=1.0 / E,
            )
            nc.vector.reciprocal(out=std, in_=std)                # rstd
            negmu = misc.tile([P, 1], f32, tag="negmu", name="negmu")
            nc.vector.tensor_scalar(
                out=negmu, in0=stat[:, 0:1], scalar1=-1.0 / E, scalar2=None,
                op0=mybir.AluOpType.mult)                         # -mean
            # (x - mu) * rstd on the otherwise-idle Pool engine
            nc.gpsimd.tensor_scalar(
                out=out_sb, in0=out_sb, scalar1=negmu[:, 0:1],
                scalar2=std[:, 0:1],
                op0=mybir.AluOpType.add, op1=mybir.AluOpType.mult,
            )
            if not skip_affine:
                nc.vector.tensor_mul(out=out_sb, in0=out_sb, in1=lngb)
                nc.gpsimd.tensor_add(out=out_sb, in0=out_sb, in1=lnbb)
            dma.dma_start(out=d_out[nsl, :], in_=out_sb)

        pq, pqc, php, ppts = prevq.pop(0)
        ctx_norm(pq, pqc, php, ppts)
        pq, pqc, php, ppts = prevq.pop(0)
        ctx_norm(pq, pqc, php, ppts)
        for nt in range(ET):
            tail_nt(nt)

    nc.compile()
    return nc


def _get_nc(skip_affine=False, skip_bias=False):
    key = ("nc", skip_affine, skip_bias)
    if key not in _cache:
        _cache[key] = _build_nc(skip_affine, skip_bias)
    return _cache[key]


def kernel(x, Wq, bq, Wk, bk, Wv, bv, Wo, bo, ln_g, ln_b, _trace=False, _tmpdir=None):
    from concourse.bass_utils import run_bass_kernel_spmd

    x = np.asarray(x, np.float32)

    # e_out permutation for the quad-DR scores layout
    m = np.arange(P)
    eo = np.empty((NQUAD, 2, P), np.int64)
    for q in range(NQUAD):
        for pi in range(2):
            eo[q, pi] = 64 * (4 * q + m // 32) + 32 * pi + (m % 32)

    def shuffle_w_qk(W):
        # W.T [e_in, e_out] -> [quad, p, plane, t, m]; e_in = t*128 + p
        wT = np.asarray(W, np.float32).T.reshape(ET, P, E)  # [t, p, e_out]
        out = np.empty((NQUAD, P, 2, ET, P), np.float32)
        for q in range(NQUAD):
            for pi in range(2):
                out[q, :, pi, :, :] = wT[:, :, eo[q, pi]].transpose(1, 0, 2)
        return np.ascontiguousarray(out).astype(FP8)

    def perm_bias(b):
        b = np.asarray(b, np.float32)
        # bqs tile loads "(t p) -> p t": vec[blk*128 + m] = b[e_out(blk, m)]
        vec = np.empty(E, np.float32)
        for q in range(NQUAD):
            for pi in range(2):
                vec[(2 * q + pi) * P + m] = b[eo[q, pi]]
        return vec

    wqR = shuffle_w_qk(Wq)
    wkR = shuffle_w_qk(Wk)
    # CTX_SCALE folded into the V weights: ctx PSUM comes out pre-scaled by
    # 16 while the ones-column denominator stays exact
    wvT = np.ascontiguousarray(
        np.asarray(Wv, np.float32).T * CTX_SCALE).astype(FP8)
    woT = np.ascontiguousarray(np.asarray(Wo, np.float32).T).astype(FP8)
    vecs = {
        "bq": perm_bias(bq), "bk": perm_bias(bk),
        "bv": np.asarray(bv, np.float32) * CTX_SCALE,
        "lng": np.asarray(ln_g, np.float32), "lnb": np.asarray(ln_b, np.float32),
    }
    bo_f = np.asarray(bo, np.float32)

    in_maps = []
    for c in range(NCORES):
        b, half = c // 2, c % 2
        xbT = np.ascontiguousarray(x[b].T).astype(FP8)
        in_maps.append({
            "xkT": xbT,
            "xqT": np.ascontiguousarray(xbT[:, half * NQ:(half + 1) * NQ]),
            # residual rows with the out-proj bias pre-added (saves a DVE pass)
            "xq": x[b, half * NQ:(half + 1) * NQ, :] + bo_f,
            "wqR": wqR, "wkR": wkR, "wvT": wvT, "woT": woT,
            **vecs,
        })

    # ln_g == 1 / ln_b == 0 make the LN affine step an exact no-op; build the
    # specialized kernel for that case (general path kept as fallback)
    skip_affine = bool(
        np.all(np.asarray(ln_g) == 1.0) and np.all(np.asarray(ln_b) == 0.0))
    skip_bias = bool(
        np.all(np.asarray(bq) == 0.0) and np.all(np.asarray(bk) == 0.0)
        and np.all(np.asarray(bv) == 0.0))
    if skip_bias:
        for mm in in_maps:
            del mm["bq"], mm["bk"], mm["bv"]
    nc = _get_nc(skip_affine, skip_bias)
    _cache["last_nc"] = nc
    res = run_bass_kernel_spmd(
        nc, in_maps, list(range(NCORES)), trace=_trace, tmpdir=_tmpdir
    )
    out = np.empty((B, S, E), np.float32)
    for c in range(NCORES):
        b, half = c // 2, c % 2
        out[b, half * NQ:(half + 1) * NQ, :] = res.results[c]["out"]
    if _trace:
        _cache["last_result"] = res
    return out
